# revision 3
# baseline (speedup 1.0000x reference)
"""EdgeConv message-passing kernel for 8 Trainium2 NeuronCores.

Strategy (pair-split + padded slot schedule + multi-queue fp16 dma_gather):
  - Queries are range-partitioned into 4 groups; refs are split into 2 halves
    (so local table indices fit int16 for dma_gather). Core c handles query
    group c>>1 and ref half c&1: its edges are those with e_query in the
    group and e_ref in the half.
  - All BatchNorms are affine at inference and fold into the weights. The
    per-edge pre-relu feature is computed by ONE block-diagonal matmul:
        z = Wfull^T @ [ref_xyz | ref_feat | 1 | q_xyz | 0...]
    where the table rows carry [ref_xyz | ref_feat | 1] in fp16 and the
    query xyz is injected on-chip into the gathered columns (units 20-22).
        h = z_relu @ W1', out[q] = relu(max_e h + c1) (empty -> 0)
  - Per core, queries are degree-sorted into tiles of 128; each tile has
    K_t slots per query (K_t = max degree in tile; padding repeats a real
    edge - idempotent under max). Edge rows are fetched with dma_gather
    (256B fp16 elements) round-robin over 4 SWDGE queues so several
    gathers' DMA rings drain concurrently (~4x the serial gather rate).
  - On-chip per gather group: DVE 32x32 block-transposes put units on
    partitions in 4 query-blocks; the Wfull matmul + scalar relu produce
    e (fp16); the W1 matmul + strided reduce_max fold slots; bias+relu and
    a final block-transpose produce the [128, 32] output rows.
  - Host side does only data movement and trivial combines: input packing /
    permutation, inverse permutation of output rows, zeroing of empty
    segments, and the pairwise max between the two ref-halves of each query
    group (the all-reduce-max of per-shard segment_max partials).
"""
import numpy as np

import concourse.bass as bass
import concourse.tile as tile
from concourse import bacc, mybir

EPS = 1e-3
P = 128
EL = 128             # fp16 units per table row (256 B dma_gather element)
MAX_GATHER_IDX = 6144
BATCH_SLOTS = 16     # slots per psum batch (16*32 = 512 = one PSUM bank)
NQUEUES = 4


def _fold_weights(inputs):
    f = np.float32
    s0 = inputs["bn0_g"] / np.sqrt(inputs["bn0_v"] + EPS)
    t0 = inputs["bn0_b"] - inputs["bn0_m"] * s0
    sf = inputs["bnf_g"] / np.sqrt(inputs["bnf_v"] + EPS)
    tf = inputs["bnf_b"] - inputs["bnf_m"] * sf
    s1 = inputs["bn1_g"] / np.sqrt(inputs["bn1_v"] + EPS)
    t1 = inputs["bn1_b"] - inputs["bn1_m"] * s1

    # Wfull rows: 0-2 ref_xyz, 3-18 ref_feat, 19 const-1, 20-22 -query_xyz
    Wf = np.zeros((32, 32), f)
    Wf[0:3] = inputs["w_pos"] * s0
    Wf[3:19] = inputs["w_feat"] * sf
    Wf[19] = (t0 + tf).astype(f)
    Wf[20:23] = -(inputs["w_pos"] * s0)
    W1 = (inputs["w1"] * s1).astype(f)
    c1 = (inputs["b1"] * s1 + t1).astype(f)

    def bd(w, dtype):
        out = np.zeros((P, P), dtype)
        for b in range(4):
            out[b * 32:(b + 1) * 32, b * 32:(b + 1) * 32] = w
        return out

    return {
        "wfull": bd(Wf, np.float16),
        "w1b": bd(W1, np.float16),
        "c14": np.tile(c1, 4).reshape(P, 1).astype(f),
    }


def _plan(inputs):
    """Host-side partitioning: per-core edge slot schedules (int bookkeeping)."""
    e_ref = np.asarray(inputs["e_ref"]).astype(np.int64)
    e_query = np.asarray(inputs["e_query"]).astype(np.int64)
    n_ref = inputs["ref_bxyz"].shape[0]
    n_q = inputs["query_bxyz"].shape[0]
    half = (n_ref + 1) // 2
    assert half <= 32767, "local table indices must fit int16"
    qg = (n_q + 3) // 4                      # queries per group
    qg_pad = ((qg + P - 1) // P) * P
    n_tiles = qg_pad // P
    n_dummy = qg_pad - qg

    cores = []
    for g in range(4):
        qlo, qhi = g * qg, min((g + 1) * qg, n_q)
        for h in range(2):
            m = (e_query >= qlo) & (e_query < qhi) & \
                (e_ref >= h * half) & (e_ref < min((h + 1) * half, n_ref))
            er = (e_ref[m] - h * half).astype(np.int64)
            eq = (e_query[m] - qlo).astype(np.int64)
            nq_local = qhi - qlo
            deg = np.bincount(eq, minlength=qg)
            order = np.argsort(eq, kind="stable")
            er_s = er[order]
            ptr = np.zeros(qg + 1, np.int64)
            np.cumsum(deg, out=ptr[1:])
            perm = np.argsort(deg, kind="stable")      # ascending degree
            qrow = np.full(qg_pad, -1, np.int64)
            qrow[n_dummy:] = perm
            degrow = np.zeros(qg_pad, np.int64)
            degrow[n_dummy:] = deg[perm]
            ptrrow = np.zeros(qg_pad, np.int64)
            ptrrow[n_dummy:] = ptr[perm]
            kt = degrow.reshape(n_tiles, P).max(axis=1)
            kt = np.maximum(kt, 1)
            cores.append({
                "g": g, "h": h, "qlo": qlo, "nq_local": nq_local,
                "er_s": er_s, "deg": deg, "qrow": qrow,
                "degrow": degrow, "ptrrow": ptrrow, "kt": kt,
            })

    # shared slot schedule across the 8 SPMD cores
    ksched = np.max(np.stack([c["kt"] for c in cores]), axis=0)

    # gather groups: consecutive tiles, <= MAX_GATHER_IDX indices each
    groups = []
    t = 0
    while t < n_tiles:
        t0_, n = t, 0
        while t < n_tiles and n + P * int(ksched[t]) <= MAX_GATHER_IDX:
            n += P * int(ksched[t])
            t += 1
        assert t > t0_, f"tile {t} alone exceeds MAX_GATHER_IDX"
        groups.append((t0_, t, n))

    meta = {
        "half": half, "qg": qg, "qg_pad": qg_pad, "n_tiles": n_tiles,
        "n_dummy": n_dummy, "ksched": ksched, "groups": groups, "n_q": n_q,
        "n_ref": n_ref,
    }
    return cores, meta


def _build_core_arrays(core, meta, inputs):
    """idx_all [128, TOT/16] int16, qx [128, n_tiles*3] f16, table [half, EL] f16."""
    half, qg_pad, n_tiles = meta["half"], meta["qg_pad"], meta["n_tiles"]
    ksched, groups = meta["ksched"], meta["groups"]
    n_ref = meta["n_ref"]
    er_s, degrow, ptrrow, qrow = (core["er_s"], core["degrow"],
                                  core["ptrrow"], core["qrow"])

    # per-tile [128, K] local table indices (pad: repeat edges cyclically)
    idx_tiles = []
    for t in range(n_tiles):
        rows = slice(t * P, (t + 1) * P)
        K = int(ksched[t])
        d = np.maximum(degrow[rows], 1)[:, None]
        j = np.arange(K)[None, :]
        pos = ptrrow[rows][:, None] + (j % d)
        if er_s.size:
            it = er_s[np.minimum(pos, er_s.size - 1)]
        else:
            it = np.zeros((P, K), np.int64)
        it = np.where(degrow[rows][:, None] > 0, it, 0)
        idx_tiles.append(it.astype(np.int16))

    # per gather group: flat index order j = s*128 + p (s = slot within group)
    wrapped = []
    for (ta, tb, nidx) in groups:
        blocks = np.concatenate([idx_tiles[t].T for t in range(ta, tb)], axis=0)
        flat = blocks.reshape(-1)                       # [nidx], j = s*128+p
        w = np.ascontiguousarray(flat.reshape(-1, 16).T)  # [16, nidx/16]
        wrapped.append(np.tile(w, (8, 1)))              # [128, nidx/16]
    idx_all = np.concatenate(wrapped, axis=1)

    # query xyz per tile-row, packed [128, n_tiles*32]: cols 20-22 of each
    # 32-col tile block carry xyz; the rest are zero (so the transposed tile
    # adds nothing on the table-data partition rows).
    qx = np.zeros((qg_pad, 32), np.float16)
    valid = qrow >= 0
    qx[valid, 20:23] = np.asarray(inputs["query_bxyz"])[core["qlo"] + qrow[valid],
                                                        1:4].astype(np.float16)
    qx_img = np.ascontiguousarray(
        qx.reshape(n_tiles, P, 32).transpose(1, 0, 2).reshape(P, n_tiles * 32))

    tab = np.zeros((half, EL), np.float16)
    lo = core["h"] * half
    hi = min(lo + half, n_ref)
    tab[:hi - lo, 0:3] = np.asarray(inputs["ref_bxyz"])[lo:hi, 1:4] \
        .astype(np.float16)
    tab[:hi - lo, 3:19] = np.asarray(inputs["ref_feat"])[lo:hi] \
        .astype(np.float16)
    tab[:hi - lo, 19] = 1.0
    return idx_all, qx_img, tab


def _build_program(meta):
    f32 = mybir.dt.float32
    f16 = mybir.dt.float16
    i16 = mybir.dt.int16
    half, qg_pad, n_tiles = meta["half"], meta["qg_pad"], meta["n_tiles"]
    ksched, groups = meta["ksched"], meta["groups"]
    tot16 = sum(n for (_, _, n) in groups) // 16
    max_blk = max(n for (_, _, n) in groups) // P

    nc = bacc.Bacc("TRN2", num_devices=8, num_swdge_queues=NQUEUES)
    table_d = nc.dram_tensor("table", [half, EL], f16, kind="ExternalInput")
    idx_d = nc.dram_tensor("idx", [P, tot16], i16, kind="ExternalInput")
    qx_d = nc.dram_tensor("qx", [P, n_tiles * 32], f16, kind="ExternalInput")
    consts = {}
    for name, shape, dt in [("wfull", [P, P], f16), ("w1b", [P, P], f16),
                            ("c14", [P, 1], f32)]:
        consts[name] = nc.dram_tensor(name, shape, dt, kind="ExternalInput")
    out_d = nc.dram_tensor("out", [qg_pad, 32], f32, kind="ExternalOutput")

    with tile.TileContext(nc) as tc:
        with tc.tile_pool(name="const", bufs=1) as cp, \
             tc.tile_pool(name="ipool", bufs=2) as ip, \
             tc.tile_pool(name="gpool", bufs=4) as gp, \
             tc.tile_pool(name="tpool", bufs=3) as tp, \
             tc.tile_pool(name="epool", bufs=2) as ep, \
             tc.tile_pool(name="spool", bufs=3) as sp, \
             tc.tile_pool(name="zps", bufs=3, space="PSUM") as zps, \
             tc.tile_pool(name="hps", bufs=3, space="PSUM") as hps:
            ct = {}
            for name, shape, dt in [("wfull", [P, P], f16),
                                    ("w1b", [P, P], f16),
                                    ("c14", [P, 1], f32)]:
                ct[name] = cp.tile(shape, dt, tag=name, name=name + "_t")
                nc.sync.dma_start(out=ct[name][:], in_=consts[name][:])
            qxall = cp.tile([P, n_tiles * 32], f16, tag="qxall")
            nc.sync.dma_start(out=qxall[:], in_=qx_d[:])
            # per-tile 32-block transpose: xyz lands on partition rows 20-22
            qxT = cp.tile([P, n_tiles * 32], f16, tag="qxT")
            nc.vector.transpose(out=qxT[:], in_=qxall[:])
            out_stage = cp.tile([P, n_tiles * 32], f32)

            off16 = 0
            for gi, (ta, tb, nidx) in enumerate(groups):
                n16 = nidx // 16
                nblk = nidx // P
                idx_t = ip.tile([P, MAX_GATHER_IDX // 16], i16, tag="idx")
                nc.sync.dma_start(out=idx_t[:, :n16],
                                  in_=idx_d[:, off16:off16 + n16])
                off16 += n16
                g_t = gp.tile([P, max_blk * EL], f16, tag="G")
                nc.gpsimd.dma_gather(
                    out_ap=g_t[:, :nblk * EL].rearrange("p (c e) -> p c e",
                                                        e=EL),
                    in_ap=table_d[:],
                    idxs_ap=idx_t[:, :n16],
                    num_idxs=nidx,
                    num_idxs_reg=nidx,
                    elem_size=EL,
                    single_packet=False,
                    queue_num=gi % NQUEUES,
                )

                # pass 1: z matmul (+ query-term accumulate) + relu -> e16.
                # Batches are tile-aligned so the broadcast query tile is
                # constant within each batch.
                g3 = g_t[:].rearrange("p (s u) -> p s u", u=EL)
                e16 = ep.tile([P, max_blk * 32], f16, tag="e")
                kofs = 0
                for t in range(ta, tb):
                    K = int(ksched[t])
                    nb = (K + BATCH_SLOTS - 1) // BATCH_SLOTS
                    s0 = 0
                    for b in range(nb):
                        bs = (K - s0) // (nb - b)
                        o = kofs + s0
                        tr = tp.tile([P, BATCH_SLOTS * 32], f16, tag="tr")
                        nc.vector.transpose(
                            out=tr[:, :bs * 32].rearrange("p (s u) -> p s u",
                                                          u=32),
                            in_=g3[:, o:o + bs, 0:32])
                        psum_z = zps.tile([P, BATCH_SLOTS * 32], f32, tag="z")
                        nc.tensor.matmul(psum_z[:, :bs * 32],
                                         lhsT=ct["wfull"][:],
                                         rhs=tr[:, :bs * 32],
                                         start=True, stop=False)
                        qb = qxT[:, t * 32:(t + 1) * 32] \
                            .rearrange("p (k u) -> p k u", k=1) \
                            .to_broadcast([P, bs, 32])
                        nc.tensor.matmul(psum_z[:, :bs * 32],
                                         lhsT=ct["wfull"][:], rhs=qb,
                                         start=False, stop=True)
                        nc.scalar.activation(e16[:, o * 32:(o + bs) * 32],
                                             psum_z[:, :bs * 32],
                                             mybir.ActivationFunctionType.Relu)
                        s0 += bs
                    kofs += K

                # pass 2: h matmul + per-tile slot reduce
                kofs = 0
                for t in range(ta, tb):
                    K = int(ksched[t])
                    acc = sp.tile([P, 32], f32, tag="acc")
                    nb = (K + BATCH_SLOTS - 1) // BATCH_SLOTS
                    s0 = 0
                    for b in range(nb):
                        bs = (K - s0) // (nb - b)
                        psum_h = hps.tile([P, BATCH_SLOTS * 32], f32, tag="h")
                        nc.tensor.matmul(
                            psum_h[:, :bs * 32], lhsT=ct["w1b"][:],
                            rhs=e16[:, (kofs + s0) * 32:(kofs + s0 + bs) * 32],
                            start=True, stop=True)
                        bmax_in = psum_h[:, :bs * 32].rearrange(
                            "p (s f) -> p f s", s=bs)
                        if b == 0:
                            nc.vector.reduce_max(out=acc[:], in_=bmax_in,
                                                 axis=mybir.AxisListType.X)
                        else:
                            bmax = sp.tile([P, 32], f32, tag="bmax")
                            nc.vector.reduce_max(out=bmax[:], in_=bmax_in,
                                                 axis=mybir.AxisListType.X)
                            nc.vector.tensor_tensor(out=acc[:], in0=acc[:],
                                                    in1=bmax[:],
                                                    op=mybir.AluOpType.max)
                        s0 += bs
                    # bias + relu, then 32-block transpose to [128q, 32f]
                    outT = sp.tile([P, 32], f32, tag="outT")
                    nc.scalar.activation(outT[:], acc[:],
                                         mybir.ActivationFunctionType.Relu,
                                         bias=ct["c14"][:, 0:1])
                    nc.vector.transpose(
                        out=out_stage[:, t * 32:(t + 1) * 32], in_=outT[:])
                    kofs += K

            nc.sync.dma_start(
                out=out_d[:].rearrange("(t p) f -> p t f", p=P),
                in_=out_stage[:].rearrange("p (t f) -> p t f", f=32))
    nc.finalize()
    return nc


def prepare(inputs):
    """Returns (nc, in_maps, postprocess)."""
    folded = _fold_weights(inputs)
    cores, meta = _plan(inputs)
    nc = _build_program(meta)
    in_maps = []
    for core in cores:
        idx_all, qx_img, tab = _build_core_arrays(core, meta, inputs)
        m = {"table": tab, "idx": idx_all, "qx": qx_img}
        m.update(folded)
        in_maps.append(m)

    def post(results):
        qg, n_q = meta["qg"], meta["n_q"]
        n_dummy = meta["n_dummy"]
        parts = []
        for ci, core in enumerate(cores):
            raw = np.asarray(results[ci]["out"])            # [qg_pad, 32]
            nq_local = core["nq_local"]
            partial = np.zeros((qg, 32), np.float32)
            partial[core["qrow"][n_dummy:]] = raw[n_dummy:]
            partial[core["deg"] == 0] = 0.0
            parts.append(partial[:nq_local])
        combined = [np.maximum(parts[2 * g], parts[2 * g + 1]) for g in range(4)]
        return np.concatenate(combined, axis=0).astype(np.float32)

    return nc, in_maps, post


def kernel(**inputs):
    from concourse.bass_utils import run_bass_kernel_spmd
    nc, in_maps, post = prepare(inputs)
    res = run_bass_kernel_spmd(nc, in_maps, core_ids=list(range(8)))
    return post(res.results)


# revision 4
# speedup vs baseline: 1.9557x; 1.9557x over previous
"""EdgeConv message-passing kernel for 8 Trainium2 NeuronCores.

Strategy (pair-split + padded slot schedule + multi-queue fp16 dma_gather):
  - Queries are range-partitioned into 4 groups; refs are split into 2 halves
    (so local table indices fit int16 for dma_gather). Core c handles query
    group c>>1 and ref half c&1: its edges are those with e_query in the
    group and e_ref in the half.
  - All BatchNorms are affine at inference and fold into the weights. The
    per-edge pre-relu feature is computed by ONE block-diagonal matmul:
        z = Wfull^T @ [ref_xyz | ref_feat | 1 | q_xyz | 0...]
    where the table rows carry [ref_xyz | ref_feat | 1] in fp16 and the
    query xyz is injected on-chip into the gathered columns (units 20-22).
        h = z_relu @ W1', out[q] = relu(max_e h + c1) (empty -> 0)
  - Per core, queries are degree-sorted into tiles of 128; each tile has
    K_t slots per query (K_t = max degree in tile; padding repeats a real
    edge - idempotent under max). Edge rows are fetched with dma_gather
    (256B fp16 elements) round-robin over 4 SWDGE queues so several
    gathers' DMA rings drain concurrently (~4x the serial gather rate).
  - On-chip per gather group: DVE 32x32 block-transposes put units on
    partitions in 4 query-blocks; the Wfull matmul + scalar relu produce
    e (fp16); the W1 matmul + strided reduce_max fold slots; bias+relu and
    a final block-transpose produce the [128, 32] output rows.
  - Host side does only data movement and trivial combines: input packing /
    permutation, inverse permutation of output rows, zeroing of empty
    segments, and the pairwise max between the two ref-halves of each query
    group (the all-reduce-max of per-shard segment_max partials).
"""
import numpy as np

import concourse.bass as bass
import concourse.tile as tile
from concourse import bacc, mybir

EPS = 1e-3
P = 128
EL = 128             # fp16 units per table row (256 B dma_gather element)
MAX_GATHER_IDX = 6144
BATCH_SLOTS = 16     # slots per psum batch (16*32 = 512 = one PSUM bank)
NQUEUES = 4


def _fold_weights(inputs):
    f = np.float32
    s0 = inputs["bn0_g"] / np.sqrt(inputs["bn0_v"] + EPS)
    t0 = inputs["bn0_b"] - inputs["bn0_m"] * s0
    sf = inputs["bnf_g"] / np.sqrt(inputs["bnf_v"] + EPS)
    tf = inputs["bnf_b"] - inputs["bnf_m"] * sf
    s1 = inputs["bn1_g"] / np.sqrt(inputs["bn1_v"] + EPS)
    t1 = inputs["bn1_b"] - inputs["bn1_m"] * s1

    # Wfull rows: 0-2 ref_xyz, 3-18 ref_feat, 19 const-1, 20-22 -query_xyz
    Wf = np.zeros((32, 32), f)
    Wf[0:3] = inputs["w_pos"] * s0
    Wf[3:19] = inputs["w_feat"] * sf
    Wf[19] = (t0 + tf).astype(f)
    Wf[20:23] = -(inputs["w_pos"] * s0)
    W1 = (inputs["w1"] * s1).astype(f)
    c1 = (inputs["b1"] * s1 + t1).astype(f)

    def bd(w, dtype):
        out = np.zeros((P, P), dtype)
        for b in range(4):
            out[b * 32:(b + 1) * 32, b * 32:(b + 1) * 32] = w
        return out

    return {
        "wfull": bd(Wf, np.float16),
        "w1b": bd(W1, np.float16),
        "c14": np.tile(c1, 4).reshape(P, 1).astype(f),
    }


def _plan(inputs):
    """Host-side partitioning: per-core edge slot schedules (int bookkeeping)."""
    e_ref = np.asarray(inputs["e_ref"]).astype(np.int64)
    e_query = np.asarray(inputs["e_query"]).astype(np.int64)
    n_ref = inputs["ref_bxyz"].shape[0]
    n_q = inputs["query_bxyz"].shape[0]
    half = (n_ref + 1) // 2
    assert half <= 32767, "local table indices must fit int16"
    qg = (n_q + 3) // 4                      # queries per group
    qg_pad = ((qg + P - 1) // P) * P
    n_tiles = qg_pad // P
    n_dummy = qg_pad - qg

    cores = []
    for g in range(4):
        qlo, qhi = g * qg, min((g + 1) * qg, n_q)
        for h in range(2):
            m = (e_query >= qlo) & (e_query < qhi) & \
                (e_ref >= h * half) & (e_ref < min((h + 1) * half, n_ref))
            er = (e_ref[m] - h * half).astype(np.int64)
            eq = (e_query[m] - qlo).astype(np.int64)
            nq_local = qhi - qlo
            deg = np.bincount(eq, minlength=qg)
            order = np.argsort(eq, kind="stable")
            er_s = er[order]
            ptr = np.zeros(qg + 1, np.int64)
            np.cumsum(deg, out=ptr[1:])
            perm = np.argsort(deg, kind="stable")      # ascending degree
            qrow = np.full(qg_pad, -1, np.int64)
            qrow[n_dummy:] = perm
            degrow = np.zeros(qg_pad, np.int64)
            degrow[n_dummy:] = deg[perm]
            ptrrow = np.zeros(qg_pad, np.int64)
            ptrrow[n_dummy:] = ptr[perm]
            kt = degrow.reshape(n_tiles, P).max(axis=1)
            kt = np.maximum(kt, 1)
            cores.append({
                "g": g, "h": h, "qlo": qlo, "nq_local": nq_local,
                "er_s": er_s, "deg": deg, "qrow": qrow,
                "degrow": degrow, "ptrrow": ptrrow, "kt": kt,
            })

    # shared slot schedule across the 8 SPMD cores
    ksched = np.max(np.stack([c["kt"] for c in cores]), axis=0)

    # gather groups: consecutive tiles, <= MAX_GATHER_IDX indices each
    groups = []
    t = 0
    while t < n_tiles:
        t0_, n = t, 0
        while t < n_tiles and n + P * int(ksched[t]) <= MAX_GATHER_IDX:
            n += P * int(ksched[t])
            t += 1
        assert t > t0_, f"tile {t} alone exceeds MAX_GATHER_IDX"
        groups.append((t0_, t, n))

    meta = {
        "half": half, "qg": qg, "qg_pad": qg_pad, "n_tiles": n_tiles,
        "n_dummy": n_dummy, "ksched": ksched, "groups": groups, "n_q": n_q,
        "n_ref": n_ref,
    }
    return cores, meta


def _build_core_arrays(core, meta, inputs):
    """idx_all [128, TOT/16] int16, qx [128, n_tiles*3] f16, table [half, EL] f16."""
    half, qg_pad, n_tiles = meta["half"], meta["qg_pad"], meta["n_tiles"]
    ksched, groups = meta["ksched"], meta["groups"]
    n_ref = meta["n_ref"]
    er_s, degrow, ptrrow, qrow = (core["er_s"], core["degrow"],
                                  core["ptrrow"], core["qrow"])

    # per-tile [128, K] local table indices (pad: repeat edges cyclically)
    idx_tiles = []
    for t in range(n_tiles):
        rows = slice(t * P, (t + 1) * P)
        K = int(ksched[t])
        d = np.maximum(degrow[rows], 1)[:, None]
        j = np.arange(K)[None, :]
        pos = ptrrow[rows][:, None] + (j % d)
        if er_s.size:
            it = er_s[np.minimum(pos, er_s.size - 1)]
        else:
            it = np.zeros((P, K), np.int64)
        it = np.where(degrow[rows][:, None] > 0, it, 0)
        idx_tiles.append(it.astype(np.int16))

    # per gather group: flat index order j = s*128 + p (s = slot within group)
    wrapped = []
    for (ta, tb, nidx) in groups:
        blocks = np.concatenate([idx_tiles[t].T for t in range(ta, tb)], axis=0)
        flat = blocks.reshape(-1)                       # [nidx], j = s*128+p
        w = np.ascontiguousarray(flat.reshape(-1, 16).T)  # [16, nidx/16]
        wrapped.append(np.tile(w, (8, 1)))              # [128, nidx/16]
    idx_all = np.concatenate(wrapped, axis=1)

    # query xyz per tile-row, packed [128, n_tiles*32]: cols 20-22 of each
    # 32-col tile block carry xyz; the rest are zero (so the transposed tile
    # adds nothing on the table-data partition rows).
    qx = np.zeros((qg_pad, 32), np.float16)
    valid = qrow >= 0
    qx[valid, 20:23] = np.asarray(inputs["query_bxyz"])[core["qlo"] + qrow[valid],
                                                        1:4].astype(np.float16)
    qx_img = np.ascontiguousarray(
        qx.reshape(n_tiles, P, 32).transpose(1, 0, 2).reshape(P, n_tiles * 32))

    tab = np.zeros((half, EL), np.float16)
    lo = core["h"] * half
    hi = min(lo + half, n_ref)
    tab[:hi - lo, 0:3] = np.asarray(inputs["ref_bxyz"])[lo:hi, 1:4] \
        .astype(np.float16)
    tab[:hi - lo, 3:19] = np.asarray(inputs["ref_feat"])[lo:hi] \
        .astype(np.float16)
    tab[:hi - lo, 19] = 1.0
    return idx_all, qx_img, tab


def _build_program(meta):
    f32 = mybir.dt.float32
    f16 = mybir.dt.float16
    i16 = mybir.dt.int16
    half, qg_pad, n_tiles = meta["half"], meta["qg_pad"], meta["n_tiles"]
    ksched, groups = meta["ksched"], meta["groups"]
    tot16 = sum(n for (_, _, n) in groups) // 16
    max_blk = max(n for (_, _, n) in groups) // P

    nc = bacc.Bacc("TRN2", num_devices=8, num_swdge_queues=NQUEUES)
    table_d = nc.dram_tensor("table", [half, EL], f16, kind="ExternalInput")
    idx_d = nc.dram_tensor("idx", [P, tot16], i16, kind="ExternalInput")
    qx_d = nc.dram_tensor("qx", [P, n_tiles * 32], f16, kind="ExternalInput")
    consts = {}
    for name, shape, dt in [("wfull", [P, P], f16), ("w1b", [P, P], f16),
                            ("c14", [P, 1], f32)]:
        consts[name] = nc.dram_tensor(name, shape, dt, kind="ExternalInput")
    out_d = nc.dram_tensor("out", [qg_pad, 32], f32, kind="ExternalOutput")

    with tile.TileContext(nc) as tc:
        with tc.tile_pool(name="const", bufs=1) as cp, \
             tc.tile_pool(name="ipool", bufs=6) as ip, \
             tc.tile_pool(name="gpool", bufs=6) as gp, \
             tc.tile_pool(name="tpool", bufs=3) as tp, \
             tc.tile_pool(name="epool", bufs=3) as ep, \
             tc.tile_pool(name="spool", bufs=3) as sp, \
             tc.tile_pool(name="zps", bufs=3, space="PSUM") as zps, \
             tc.tile_pool(name="hps", bufs=3, space="PSUM") as hps:
            ct = {}
            for name, shape, dt in [("wfull", [P, P], f16),
                                    ("w1b", [P, P], f16),
                                    ("c14", [P, 1], f32)]:
                ct[name] = cp.tile(shape, dt, tag=name, name=name + "_t")
                nc.sync.dma_start(out=ct[name][:], in_=consts[name][:])
            qxall = cp.tile([P, n_tiles * 32], f16, tag="qxall")
            nc.sync.dma_start(out=qxall[:], in_=qx_d[:])
            # per-tile 32-block transpose: xyz lands on partition rows 20-22
            qxT = cp.tile([P, n_tiles * 32], f16, tag="qxT")
            nc.vector.transpose(out=qxT[:], in_=qxall[:])
            out_stage = cp.tile([P, n_tiles * 32], f32)

            off16 = 0
            for gi, (ta, tb, nidx) in enumerate(groups):
                n16 = nidx // 16
                nblk = nidx // P
                idx_t = ip.tile([P, MAX_GATHER_IDX // 16], i16, tag="idx")
                nc.sync.dma_start(out=idx_t[:, :n16],
                                  in_=idx_d[:, off16:off16 + n16])
                off16 += n16
                g_t = gp.tile([P, max_blk * EL], f16, tag="G")
                nc.gpsimd.dma_gather(
                    out_ap=g_t[:, :nblk * EL].rearrange("p (c e) -> p c e",
                                                        e=EL),
                    in_ap=table_d[:],
                    idxs_ap=idx_t[:, :n16],
                    num_idxs=nidx,
                    num_idxs_reg=nidx,
                    elem_size=EL,
                    single_packet=False,
                    queue_num=gi % NQUEUES,
                )

                # pass 1: z matmul (+ query-term accumulate) + relu -> e16.
                # Batches are tile-aligned so the broadcast query tile is
                # constant within each batch.
                g3 = g_t[:].rearrange("p (s u) -> p s u", u=EL)
                e16 = ep.tile([P, max_blk * 32], f16, tag="e")
                kofs = 0
                for t in range(ta, tb):
                    K = int(ksched[t])
                    nb = (K + BATCH_SLOTS - 1) // BATCH_SLOTS
                    s0 = 0
                    for b in range(nb):
                        bs = (K - s0) // (nb - b)
                        o = kofs + s0
                        tr = tp.tile([P, BATCH_SLOTS * 32], f16, tag="tr")
                        nc.vector.transpose(
                            out=tr[:, :bs * 32].rearrange("p (s u) -> p s u",
                                                          u=32),
                            in_=g3[:, o:o + bs, 0:32])
                        psum_z = zps.tile([P, BATCH_SLOTS * 32], f32, tag="z")
                        nc.tensor.matmul(psum_z[:, :bs * 32],
                                         lhsT=ct["wfull"][:],
                                         rhs=tr[:, :bs * 32],
                                         start=True, stop=False)
                        qb = qxT[:, t * 32:(t + 1) * 32] \
                            .rearrange("p (k u) -> p k u", k=1) \
                            .to_broadcast([P, bs, 32])
                        nc.tensor.matmul(psum_z[:, :bs * 32],
                                         lhsT=ct["wfull"][:], rhs=qb,
                                         start=False, stop=True)
                        nc.scalar.activation(e16[:, o * 32:(o + bs) * 32],
                                             psum_z[:, :bs * 32],
                                             mybir.ActivationFunctionType.Relu)
                        s0 += bs
                    kofs += K

                # pass 2: h matmul + per-tile slot reduce
                kofs = 0
                for t in range(ta, tb):
                    K = int(ksched[t])
                    acc = sp.tile([P, 32], f32, tag="acc")
                    nb = (K + BATCH_SLOTS - 1) // BATCH_SLOTS
                    s0 = 0
                    for b in range(nb):
                        bs = (K - s0) // (nb - b)
                        psum_h = hps.tile([P, BATCH_SLOTS * 32], f32, tag="h")
                        nc.tensor.matmul(
                            psum_h[:, :bs * 32], lhsT=ct["w1b"][:],
                            rhs=e16[:, (kofs + s0) * 32:(kofs + s0 + bs) * 32],
                            start=True, stop=True)
                        bmax_in = psum_h[:, :bs * 32].rearrange(
                            "p (s f) -> p f s", s=bs)
                        if b == 0:
                            nc.vector.reduce_max(out=acc[:], in_=bmax_in,
                                                 axis=mybir.AxisListType.X)
                        else:
                            bmax = sp.tile([P, 32], f32, tag="bmax")
                            nc.vector.reduce_max(out=bmax[:], in_=bmax_in,
                                                 axis=mybir.AxisListType.X)
                            nc.vector.tensor_tensor(out=acc[:], in0=acc[:],
                                                    in1=bmax[:],
                                                    op=mybir.AluOpType.max)
                        s0 += bs
                    # bias + relu, then 32-block transpose to [128q, 32f]
                    outT = sp.tile([P, 32], f32, tag="outT")
                    nc.scalar.activation(outT[:], acc[:],
                                         mybir.ActivationFunctionType.Relu,
                                         bias=ct["c14"][:, 0:1])
                    nc.vector.transpose(
                        out=out_stage[:, t * 32:(t + 1) * 32], in_=outT[:])
                    kofs += K

            nc.sync.dma_start(
                out=out_d[:].rearrange("(t p) f -> p t f", p=P),
                in_=out_stage[:].rearrange("p (t f) -> p t f", f=32))
    nc.finalize()
    return nc


def prepare(inputs):
    """Returns (nc, in_maps, postprocess)."""
    folded = _fold_weights(inputs)
    cores, meta = _plan(inputs)
    nc = _build_program(meta)
    in_maps = []
    for core in cores:
        idx_all, qx_img, tab = _build_core_arrays(core, meta, inputs)
        m = {"table": tab, "idx": idx_all, "qx": qx_img}
        m.update(folded)
        in_maps.append(m)

    def post(results):
        qg, n_q = meta["qg"], meta["n_q"]
        n_dummy = meta["n_dummy"]
        parts = []
        for ci, core in enumerate(cores):
            raw = np.asarray(results[ci]["out"])            # [qg_pad, 32]
            nq_local = core["nq_local"]
            partial = np.zeros((qg, 32), np.float32)
            partial[core["qrow"][n_dummy:]] = raw[n_dummy:]
            partial[core["deg"] == 0] = 0.0
            parts.append(partial[:nq_local])
        combined = [np.maximum(parts[2 * g], parts[2 * g + 1]) for g in range(4)]
        return np.concatenate(combined, axis=0).astype(np.float32)

    return nc, in_maps, post


def kernel(**inputs):
    from concourse.bass_utils import run_bass_kernel_spmd
    nc, in_maps, post = prepare(inputs)
    res = run_bass_kernel_spmd(nc, in_maps, core_ids=list(range(8)))
    return post(res.results)


# revision 5
# speedup vs baseline: 2.3528x; 1.2031x over previous
"""EdgeConv message-passing kernel for 8 Trainium2 NeuronCores.

Strategy (pair-split + padded slot schedule + multi-queue fp16 dma_gather):
  - Queries are range-partitioned into 4 groups; refs are split into 2 halves
    (so local table indices fit int16 for dma_gather). Core c handles query
    group c>>1 and ref half c&1: its edges are those with e_query in the
    group and e_ref in the half.
  - All BatchNorms are affine at inference and fold into the weights. The
    per-edge pre-relu feature is computed by ONE block-diagonal matmul:
        z = Wfull^T @ [ref_xyz | ref_feat | 1 | q_xyz | 0...]
    where the table rows carry [ref_xyz | ref_feat | 1] in fp16 and the
    query xyz is injected on-chip into the gathered columns (units 20-22).
        h = z_relu @ W1', out[q] = relu(max_e h + c1) (empty -> 0)
  - Per core, queries are degree-sorted into tiles of 128; each tile has
    K_t slots per query (K_t = max degree in tile; padding repeats a real
    edge - idempotent under max). Edge rows are fetched with dma_gather
    (256B fp16 elements) round-robin over 4 SWDGE queues so several
    gathers' DMA rings drain concurrently (~4x the serial gather rate).
  - On-chip per gather group: DVE 32x32 block-transposes put units on
    partitions in 4 query-blocks; the Wfull matmul + scalar relu produce
    e (fp16); the W1 matmul + strided reduce_max fold slots; bias+relu and
    a final block-transpose produce the [128, 32] output rows.
  - Host side does only data movement and trivial combines: input packing /
    permutation, inverse permutation of output rows, zeroing of empty
    segments, and the pairwise max between the two ref-halves of each query
    group (the all-reduce-max of per-shard segment_max partials).
"""
import numpy as np

import concourse.bass as bass
import concourse.tile as tile
from concourse import bacc, mybir

EPS = 1e-3
P = 128
EL = 128             # fp16 units per table row (256 B dma_gather element)
MAX_GATHER_IDX = 6144
BATCH_SLOTS = 16     # slots per psum batch (16*32 = 512 = one PSUM bank)
NQUEUES = 4


def _fold_weights(inputs):
    f = np.float32
    s0 = inputs["bn0_g"] / np.sqrt(inputs["bn0_v"] + EPS)
    t0 = inputs["bn0_b"] - inputs["bn0_m"] * s0
    sf = inputs["bnf_g"] / np.sqrt(inputs["bnf_v"] + EPS)
    tf = inputs["bnf_b"] - inputs["bnf_m"] * sf
    s1 = inputs["bn1_g"] / np.sqrt(inputs["bn1_v"] + EPS)
    t1 = inputs["bn1_b"] - inputs["bn1_m"] * s1

    # Wfull rows: 0-2 ref_xyz, 3-18 ref_feat, 19 const-1, 20-22 -query_xyz
    Wf = np.zeros((32, 32), f)
    Wf[0:3] = inputs["w_pos"] * s0
    Wf[3:19] = inputs["w_feat"] * sf
    Wf[19] = (t0 + tf).astype(f)
    Wf[20:23] = -(inputs["w_pos"] * s0)
    W1 = (inputs["w1"] * s1).astype(f)
    c1 = (inputs["b1"] * s1 + t1).astype(f)

    def bd(w, dtype):
        out = np.zeros((P, P), dtype)
        for b in range(4):
            out[b * 32:(b + 1) * 32, b * 32:(b + 1) * 32] = w
        return out

    return {
        "wfull": bd(Wf, np.float16),
        "w1b": bd(W1, np.float16),
        "c14": np.tile(c1, 4).reshape(P, 1).astype(f),
    }


def _plan(inputs):
    """Host-side partitioning: per-core edge slot schedules (int bookkeeping)."""
    e_ref = np.asarray(inputs["e_ref"]).astype(np.int64)
    e_query = np.asarray(inputs["e_query"]).astype(np.int64)
    n_ref = inputs["ref_bxyz"].shape[0]
    n_q = inputs["query_bxyz"].shape[0]
    half = (n_ref + 1) // 2
    assert half <= 32767, "local table indices must fit int16"
    qg = (n_q + 3) // 4                      # queries per group
    qg_pad = ((qg + P - 1) // P) * P
    n_tiles = qg_pad // P
    n_dummy = qg_pad - qg

    cores = []
    for g in range(4):
        qlo, qhi = g * qg, min((g + 1) * qg, n_q)
        for h in range(2):
            m = (e_query >= qlo) & (e_query < qhi) & \
                (e_ref >= h * half) & (e_ref < min((h + 1) * half, n_ref))
            er = (e_ref[m] - h * half).astype(np.int64)
            eq = (e_query[m] - qlo).astype(np.int64)
            nq_local = qhi - qlo
            deg = np.bincount(eq, minlength=qg)
            order = np.argsort(eq, kind="stable")
            er_s = er[order]
            ptr = np.zeros(qg + 1, np.int64)
            np.cumsum(deg, out=ptr[1:])
            perm = np.argsort(deg, kind="stable")      # ascending degree
            qrow = np.full(qg_pad, -1, np.int64)
            qrow[n_dummy:] = perm
            degrow = np.zeros(qg_pad, np.int64)
            degrow[n_dummy:] = deg[perm]
            ptrrow = np.zeros(qg_pad, np.int64)
            ptrrow[n_dummy:] = ptr[perm]
            kt = degrow.reshape(n_tiles, P).max(axis=1)
            kt = np.maximum(kt, 1)
            cores.append({
                "g": g, "h": h, "qlo": qlo, "nq_local": nq_local,
                "er_s": er_s, "deg": deg, "qrow": qrow,
                "degrow": degrow, "ptrrow": ptrrow, "kt": kt,
            })

    # shared slot schedule across the 8 SPMD cores
    ksched = np.max(np.stack([c["kt"] for c in cores]), axis=0)

    # gather groups: consecutive tiles, <= MAX_GATHER_IDX indices each.
    # The first NQUEUES groups are single tiles so all four SWDGE queues
    # spin up quickly instead of one full-size gather running solo.
    groups = []
    t = 0
    while t < n_tiles and len(groups) < NQUEUES:
        groups.append((t, t + 1, P * int(ksched[t])))
        t += 1
    while t < n_tiles:
        t0_, n = t, 0
        while t < n_tiles and n + P * int(ksched[t]) <= MAX_GATHER_IDX:
            n += P * int(ksched[t])
            t += 1
        assert t > t0_, f"tile {t} alone exceeds MAX_GATHER_IDX"
        groups.append((t0_, t, n))

    meta = {
        "half": half, "qg": qg, "qg_pad": qg_pad, "n_tiles": n_tiles,
        "n_dummy": n_dummy, "ksched": ksched, "groups": groups, "n_q": n_q,
        "n_ref": n_ref,
    }
    return cores, meta


def _build_core_arrays(core, meta, inputs):
    """idx_all [128, TOT/16] int16, qx [128, n_tiles*3] f16, table [half, EL] f16."""
    half, qg_pad, n_tiles = meta["half"], meta["qg_pad"], meta["n_tiles"]
    ksched, groups = meta["ksched"], meta["groups"]
    n_ref = meta["n_ref"]
    er_s, degrow, ptrrow, qrow = (core["er_s"], core["degrow"],
                                  core["ptrrow"], core["qrow"])

    # per-tile [128, K] local table indices (pad: repeat edges cyclically)
    idx_tiles = []
    for t in range(n_tiles):
        rows = slice(t * P, (t + 1) * P)
        K = int(ksched[t])
        d = np.maximum(degrow[rows], 1)[:, None]
        j = np.arange(K)[None, :]
        pos = ptrrow[rows][:, None] + (j % d)
        if er_s.size:
            it = er_s[np.minimum(pos, er_s.size - 1)]
        else:
            it = np.zeros((P, K), np.int64)
        it = np.where(degrow[rows][:, None] > 0, it, 0)
        idx_tiles.append(it.astype(np.int16))

    # per gather group: flat index order j = s*128 + p (s = slot within group)
    wrapped = []
    for (ta, tb, nidx) in groups:
        blocks = np.concatenate([idx_tiles[t].T for t in range(ta, tb)], axis=0)
        flat = blocks.reshape(-1)                       # [nidx], j = s*128+p
        w = np.ascontiguousarray(flat.reshape(-1, 16).T)  # [16, nidx/16]
        wrapped.append(np.tile(w, (8, 1)))              # [128, nidx/16]
    idx_all = np.concatenate(wrapped, axis=1)

    # query xyz per tile-row, packed [128, n_tiles*32]: cols 20-22 of each
    # 32-col tile block carry xyz; the rest are zero (so the transposed tile
    # adds nothing on the table-data partition rows).
    qx = np.zeros((qg_pad, 32), np.float16)
    valid = qrow >= 0
    qx[valid, 20:23] = np.asarray(inputs["query_bxyz"])[core["qlo"] + qrow[valid],
                                                        1:4].astype(np.float16)
    qx_img = np.ascontiguousarray(
        qx.reshape(n_tiles, P, 32).transpose(1, 0, 2).reshape(P, n_tiles * 32))

    tab = np.zeros((half, EL), np.float16)
    lo = core["h"] * half
    hi = min(lo + half, n_ref)
    tab[:hi - lo, 0:3] = np.asarray(inputs["ref_bxyz"])[lo:hi, 1:4] \
        .astype(np.float16)
    tab[:hi - lo, 3:19] = np.asarray(inputs["ref_feat"])[lo:hi] \
        .astype(np.float16)
    tab[:hi - lo, 19] = 1.0
    return idx_all, qx_img, tab


def _build_program(meta):
    f32 = mybir.dt.float32
    f16 = mybir.dt.float16
    i16 = mybir.dt.int16
    half, qg_pad, n_tiles = meta["half"], meta["qg_pad"], meta["n_tiles"]
    ksched, groups = meta["ksched"], meta["groups"]
    tot16 = sum(n for (_, _, n) in groups) // 16
    max_blk = max(n for (_, _, n) in groups) // P

    nc = bacc.Bacc("TRN2", num_devices=8, num_swdge_queues=NQUEUES)
    table_d = nc.dram_tensor("table", [half, EL], f16, kind="ExternalInput")
    idx_d = nc.dram_tensor("idx", [P, tot16], i16, kind="ExternalInput")
    qx_d = nc.dram_tensor("qx", [P, n_tiles * 32], f16, kind="ExternalInput")
    consts = {}
    for name, shape, dt in [("wfull", [P, P], f16), ("w1b", [P, P], f16),
                            ("c14", [P, 1], f32)]:
        consts[name] = nc.dram_tensor(name, shape, dt, kind="ExternalInput")
    out_d = nc.dram_tensor("out", [qg_pad, 32], f32, kind="ExternalOutput")

    with tile.TileContext(nc) as tc:
        with tc.tile_pool(name="const", bufs=1) as cp, \
             tc.tile_pool(name="ipool", bufs=6) as ip, \
             tc.tile_pool(name="gpool", bufs=6) as gp, \
             tc.tile_pool(name="tpool", bufs=3) as tp, \
             tc.tile_pool(name="epool", bufs=3) as ep, \
             tc.tile_pool(name="spool", bufs=3) as sp, \
             tc.tile_pool(name="zps", bufs=3, space="PSUM") as zps, \
             tc.tile_pool(name="hps", bufs=3, space="PSUM") as hps:
            ct = {}
            for name, shape, dt in [("wfull", [P, P], f16),
                                    ("w1b", [P, P], f16),
                                    ("c14", [P, 1], f32)]:
                ct[name] = cp.tile(shape, dt, tag=name, name=name + "_t")
                nc.sync.dma_start(out=ct[name][:], in_=consts[name][:])
            qxall = cp.tile([P, n_tiles * 32], f16, tag="qxall")
            nc.sync.dma_start(out=qxall[:], in_=qx_d[:])
            # per-tile 32-block transpose: xyz lands on partition rows 20-22
            qxT = cp.tile([P, n_tiles * 32], f16, tag="qxT")
            nc.vector.transpose(out=qxT[:], in_=qxall[:])
            out_stage = cp.tile([P, n_tiles * 32], f32)

            off16 = 0
            for gi, (ta, tb, nidx) in enumerate(groups):
                n16 = nidx // 16
                nblk = nidx // P
                idx_t = ip.tile([P, MAX_GATHER_IDX // 16], i16, tag="idx")
                nc.sync.dma_start(out=idx_t[:, :n16],
                                  in_=idx_d[:, off16:off16 + n16])
                off16 += n16
                g_t = gp.tile([P, max_blk * EL], f16, tag="G")
                nc.gpsimd.dma_gather(
                    out_ap=g_t[:, :nblk * EL].rearrange("p (c e) -> p c e",
                                                        e=EL),
                    in_ap=table_d[:],
                    idxs_ap=idx_t[:, :n16],
                    num_idxs=nidx,
                    num_idxs_reg=nidx,
                    elem_size=EL,
                    single_packet=False,
                    queue_num=gi % NQUEUES,
                )

                # pass 1: z matmul (+ query-term accumulate) + relu -> e16.
                # Batches are tile-aligned so the broadcast query tile is
                # constant within each batch.
                g3 = g_t[:].rearrange("p (s u) -> p s u", u=EL)
                e16 = ep.tile([P, max_blk * 32], f16, tag="e")
                kofs = 0
                for t in range(ta, tb):
                    K = int(ksched[t])
                    nb = (K + BATCH_SLOTS - 1) // BATCH_SLOTS
                    s0 = 0
                    for b in range(nb):
                        bs = (K - s0) // (nb - b)
                        o = kofs + s0
                        tr = tp.tile([P, BATCH_SLOTS * 32], f16, tag="tr")
                        nc.vector.transpose(
                            out=tr[:, :bs * 32].rearrange("p (s u) -> p s u",
                                                          u=32),
                            in_=g3[:, o:o + bs, 0:32])
                        psum_z = zps.tile([P, BATCH_SLOTS * 32], f32, tag="z")
                        nc.tensor.matmul(psum_z[:, :bs * 32],
                                         lhsT=ct["wfull"][:],
                                         rhs=tr[:, :bs * 32],
                                         start=True, stop=False)
                        qb = qxT[:, t * 32:(t + 1) * 32] \
                            .rearrange("p (k u) -> p k u", k=1) \
                            .to_broadcast([P, bs, 32])
                        nc.tensor.matmul(psum_z[:, :bs * 32],
                                         lhsT=ct["wfull"][:], rhs=qb,
                                         start=False, stop=True)
                        nc.scalar.activation(e16[:, o * 32:(o + bs) * 32],
                                             psum_z[:, :bs * 32],
                                             mybir.ActivationFunctionType.Relu)
                        s0 += bs
                    kofs += K

                # pass 2: h matmul + per-tile slot reduce
                kofs = 0
                for t in range(ta, tb):
                    K = int(ksched[t])
                    acc = sp.tile([P, 32], f32, tag="acc")
                    nb = (K + BATCH_SLOTS - 1) // BATCH_SLOTS
                    s0 = 0
                    for b in range(nb):
                        bs = (K - s0) // (nb - b)
                        psum_h = hps.tile([P, BATCH_SLOTS * 32], f32, tag="h")
                        nc.tensor.matmul(
                            psum_h[:, :bs * 32], lhsT=ct["w1b"][:],
                            rhs=e16[:, (kofs + s0) * 32:(kofs + s0 + bs) * 32],
                            start=True, stop=True)
                        bmax_in = psum_h[:, :bs * 32].rearrange(
                            "p (s f) -> p f s", s=bs)
                        if b == 0:
                            nc.vector.reduce_max(out=acc[:], in_=bmax_in,
                                                 axis=mybir.AxisListType.X)
                        else:
                            bmax = sp.tile([P, 32], f32, tag="bmax")
                            nc.vector.reduce_max(out=bmax[:], in_=bmax_in,
                                                 axis=mybir.AxisListType.X)
                            nc.vector.tensor_tensor(out=acc[:], in0=acc[:],
                                                    in1=bmax[:],
                                                    op=mybir.AluOpType.max)
                        s0 += bs
                    # bias + relu, then 32-block transpose to [128q, 32f]
                    outT = sp.tile([P, 32], f32, tag="outT")
                    nc.scalar.activation(outT[:], acc[:],
                                         mybir.ActivationFunctionType.Relu,
                                         bias=ct["c14"][:, 0:1])
                    nc.vector.transpose(
                        out=out_stage[:, t * 32:(t + 1) * 32], in_=outT[:])
                    kofs += K

            nc.sync.dma_start(
                out=out_d[:].rearrange("(t p) f -> p t f", p=P),
                in_=out_stage[:].rearrange("p (t f) -> p t f", f=32))
    nc.finalize()
    return nc


def prepare(inputs):
    """Returns (nc, in_maps, postprocess)."""
    folded = _fold_weights(inputs)
    cores, meta = _plan(inputs)
    nc = _build_program(meta)
    in_maps = []
    for core in cores:
        idx_all, qx_img, tab = _build_core_arrays(core, meta, inputs)
        m = {"table": tab, "idx": idx_all, "qx": qx_img}
        m.update(folded)
        in_maps.append(m)

    def post(results):
        qg, n_q = meta["qg"], meta["n_q"]
        n_dummy = meta["n_dummy"]
        parts = []
        for ci, core in enumerate(cores):
            raw = np.asarray(results[ci]["out"])            # [qg_pad, 32]
            nq_local = core["nq_local"]
            partial = np.zeros((qg, 32), np.float32)
            partial[core["qrow"][n_dummy:]] = raw[n_dummy:]
            partial[core["deg"] == 0] = 0.0
            parts.append(partial[:nq_local])
        combined = [np.maximum(parts[2 * g], parts[2 * g + 1]) for g in range(4)]
        return np.concatenate(combined, axis=0).astype(np.float32)

    return nc, in_maps, post


def kernel(**inputs):
    from concourse.bass_utils import run_bass_kernel_spmd
    nc, in_maps, post = prepare(inputs)
    res = run_bass_kernel_spmd(nc, in_maps, core_ids=list(range(8)))
    return post(res.results)


# revision 6
# speedup vs baseline: 2.4493x; 1.0410x over previous
"""EdgeConv message-passing kernel for 8 Trainium2 NeuronCores.

Strategy (pair-split + padded slot schedule + multi-queue fp16 dma_gather):
  - Queries are range-partitioned into 4 groups; refs are split into 2 halves
    (so local table indices fit int16 for dma_gather). Core c handles query
    group c>>1 and ref half c&1: its edges are those with e_query in the
    group and e_ref in the half.
  - All BatchNorms are affine at inference and fold into the weights. The
    per-edge pre-relu feature is computed by ONE block-diagonal matmul:
        z = Wfull^T @ [ref_xyz | ref_feat | 1 | q_xyz | 0...]
    where the table rows carry [ref_xyz | ref_feat | 1] in fp16 and the
    query xyz is injected on-chip into the gathered columns (units 20-22).
        h = z_relu @ W1', out[q] = relu(max_e h + c1) (empty -> 0)
  - Per core, queries are degree-sorted into tiles of 128; each tile has
    K_t slots per query (K_t = max degree in tile; padding repeats a real
    edge - idempotent under max). Edge rows are fetched with dma_gather
    (256B fp16 elements) round-robin over 4 SWDGE queues so several
    gathers' DMA rings drain concurrently (~4x the serial gather rate).
  - On-chip per gather group: DVE 32x32 block-transposes put units on
    partitions in 4 query-blocks; the Wfull matmul + scalar relu produce
    e (fp16); the W1 matmul + strided reduce_max fold slots; bias+relu and
    a final block-transpose produce the [128, 32] output rows.
  - Host side does only data movement and trivial combines: input packing /
    permutation, inverse permutation of output rows, zeroing of empty
    segments, and the pairwise max between the two ref-halves of each query
    group (the all-reduce-max of per-shard segment_max partials).
"""
import numpy as np

import concourse.bass as bass
import concourse.tile as tile
from concourse import bacc, mybir

EPS = 1e-3
P = 128
EL = 128             # fp16 units per table row (256 B dma_gather element)
MAX_GATHER_IDX = 6144
BATCH_SLOTS = 16     # slots per psum batch (16*32 = 512 = one PSUM bank)
NQUEUES = 4


def _fold_weights(inputs):
    f = np.float32
    s0 = inputs["bn0_g"] / np.sqrt(inputs["bn0_v"] + EPS)
    t0 = inputs["bn0_b"] - inputs["bn0_m"] * s0
    sf = inputs["bnf_g"] / np.sqrt(inputs["bnf_v"] + EPS)
    tf = inputs["bnf_b"] - inputs["bnf_m"] * sf
    s1 = inputs["bn1_g"] / np.sqrt(inputs["bn1_v"] + EPS)
    t1 = inputs["bn1_b"] - inputs["bn1_m"] * s1

    # Wfull rows: 0-2 ref_xyz, 3-18 ref_feat, 19 const-1, 20-22 -query_xyz
    Wf = np.zeros((32, 32), f)
    Wf[0:3] = inputs["w_pos"] * s0
    Wf[3:19] = inputs["w_feat"] * sf
    Wf[19] = (t0 + tf).astype(f)
    Wf[20:23] = -(inputs["w_pos"] * s0)
    W1 = (inputs["w1"] * s1).astype(f)
    c1 = (inputs["b1"] * s1 + t1).astype(f)

    def bd(w, dtype):
        out = np.zeros((P, P), dtype)
        for b in range(4):
            out[b * 32:(b + 1) * 32, b * 32:(b + 1) * 32] = w
        return out

    return {
        "wfull": bd(Wf, np.float16),
        "w1b": bd(W1, np.float16),
        "c14": np.tile(c1, 4).reshape(P, 1).astype(f),
    }


def _plan(inputs):
    """Host-side partitioning: per-core edge slot schedules (int bookkeeping)."""
    e_ref = np.asarray(inputs["e_ref"]).astype(np.int64)
    e_query = np.asarray(inputs["e_query"]).astype(np.int64)
    n_ref = inputs["ref_bxyz"].shape[0]
    n_q = inputs["query_bxyz"].shape[0]
    half = (n_ref + 1) // 2
    assert half <= 32767, "local table indices must fit int16"
    qg = (n_q + 3) // 4                      # queries per group
    qg_pad = ((qg + P - 1) // P) * P
    n_tiles = qg_pad // P
    n_dummy = qg_pad - qg

    cores = []
    for g in range(4):
        qlo, qhi = g * qg, min((g + 1) * qg, n_q)
        for h in range(2):
            m = (e_query >= qlo) & (e_query < qhi) & \
                (e_ref >= h * half) & (e_ref < min((h + 1) * half, n_ref))
            er = (e_ref[m] - h * half).astype(np.int64)
            eq = (e_query[m] - qlo).astype(np.int64)
            nq_local = qhi - qlo
            deg = np.bincount(eq, minlength=qg)
            order = np.argsort(eq, kind="stable")
            er_s = er[order]
            ptr = np.zeros(qg + 1, np.int64)
            np.cumsum(deg, out=ptr[1:])
            perm = np.argsort(deg, kind="stable")      # ascending degree
            qrow = np.full(qg_pad, -1, np.int64)
            qrow[n_dummy:] = perm
            degrow = np.zeros(qg_pad, np.int64)
            degrow[n_dummy:] = deg[perm]
            ptrrow = np.zeros(qg_pad, np.int64)
            ptrrow[n_dummy:] = ptr[perm]
            kt = degrow.reshape(n_tiles, P).max(axis=1)
            kt = np.maximum(kt, 1)
            cores.append({
                "g": g, "h": h, "qlo": qlo, "nq_local": nq_local,
                "er_s": er_s, "deg": deg, "qrow": qrow,
                "degrow": degrow, "ptrrow": ptrrow, "kt": kt,
            })

    # shared slot schedule across the 8 SPMD cores
    ksched = np.max(np.stack([c["kt"] for c in cores]), axis=0)

    # gather groups: consecutive tiles, <= MAX_GATHER_IDX indices each.
    # The first NQUEUES groups are single tiles so all four SWDGE queues
    # spin up quickly instead of one full-size gather running solo.
    groups = []
    t = 0
    while t < n_tiles and len(groups) < NQUEUES:
        groups.append((t, t + 1, P * int(ksched[t])))
        t += 1
    t_hi = n_tiles - 2          # last two tiles: own groups (short tail)
    while t < t_hi:
        t0_, n = t, 0
        while t < t_hi and n + P * int(ksched[t]) <= MAX_GATHER_IDX:
            n += P * int(ksched[t])
            t += 1
        assert t > t0_, f"tile {t} alone exceeds MAX_GATHER_IDX"
        groups.append((t0_, t, n))
    while t < n_tiles:
        groups.append((t, t + 1, P * int(ksched[t])))
        t += 1

    meta = {
        "half": half, "qg": qg, "qg_pad": qg_pad, "n_tiles": n_tiles,
        "n_dummy": n_dummy, "ksched": ksched, "groups": groups, "n_q": n_q,
        "n_ref": n_ref,
    }
    return cores, meta


def _build_core_arrays(core, meta, inputs):
    """idx_all [128, TOT/16] int16, qx [128, n_tiles*3] f16, table [half, EL] f16."""
    half, qg_pad, n_tiles = meta["half"], meta["qg_pad"], meta["n_tiles"]
    ksched, groups = meta["ksched"], meta["groups"]
    n_ref = meta["n_ref"]
    er_s, degrow, ptrrow, qrow = (core["er_s"], core["degrow"],
                                  core["ptrrow"], core["qrow"])

    # per-tile [128, K] local table indices (pad: repeat edges cyclically)
    idx_tiles = []
    for t in range(n_tiles):
        rows = slice(t * P, (t + 1) * P)
        K = int(ksched[t])
        d = np.maximum(degrow[rows], 1)[:, None]
        j = np.arange(K)[None, :]
        pos = ptrrow[rows][:, None] + (j % d)
        if er_s.size:
            it = er_s[np.minimum(pos, er_s.size - 1)]
        else:
            it = np.zeros((P, K), np.int64)
        it = np.where(degrow[rows][:, None] > 0, it, 0)
        idx_tiles.append(it.astype(np.int16))

    # per gather group: flat index order j = s*128 + p (s = slot within group)
    wrapped = []
    for (ta, tb, nidx) in groups:
        blocks = np.concatenate([idx_tiles[t].T for t in range(ta, tb)], axis=0)
        flat = blocks.reshape(-1)                       # [nidx], j = s*128+p
        w = np.ascontiguousarray(flat.reshape(-1, 16).T)  # [16, nidx/16]
        wrapped.append(np.tile(w, (8, 1)))              # [128, nidx/16]
    idx_all = np.concatenate(wrapped, axis=1)

    # query xyz per tile-row, packed [128, n_tiles*32]: cols 20-22 of each
    # 32-col tile block carry xyz; the rest are zero (so the transposed tile
    # adds nothing on the table-data partition rows).
    qx = np.zeros((qg_pad, 32), np.float16)
    valid = qrow >= 0
    qx[valid, 20:23] = np.asarray(inputs["query_bxyz"])[core["qlo"] + qrow[valid],
                                                        1:4].astype(np.float16)
    qx_img = np.ascontiguousarray(
        qx.reshape(n_tiles, P, 32).transpose(1, 0, 2).reshape(P, n_tiles * 32))

    tab = np.zeros((half, EL), np.float16)
    lo = core["h"] * half
    hi = min(lo + half, n_ref)
    tab[:hi - lo, 0:3] = np.asarray(inputs["ref_bxyz"])[lo:hi, 1:4] \
        .astype(np.float16)
    tab[:hi - lo, 3:19] = np.asarray(inputs["ref_feat"])[lo:hi] \
        .astype(np.float16)
    tab[:hi - lo, 19] = 1.0
    return idx_all, qx_img, tab


def _build_program(meta):
    f32 = mybir.dt.float32
    f16 = mybir.dt.float16
    i16 = mybir.dt.int16
    half, qg_pad, n_tiles = meta["half"], meta["qg_pad"], meta["n_tiles"]
    ksched, groups = meta["ksched"], meta["groups"]
    tot16 = sum(n for (_, _, n) in groups) // 16
    max_blk = max(n for (_, _, n) in groups) // P

    nc = bacc.Bacc("TRN2", num_devices=8, num_swdge_queues=NQUEUES)
    table_d = nc.dram_tensor("table", [half, EL], f16, kind="ExternalInput")
    idx_d = nc.dram_tensor("idx", [P, tot16], i16, kind="ExternalInput")
    qx_d = nc.dram_tensor("qx", [P, n_tiles * 32], f16, kind="ExternalInput")
    consts = {}
    for name, shape, dt in [("wfull", [P, P], f16), ("w1b", [P, P], f16),
                            ("c14", [P, 1], f32)]:
        consts[name] = nc.dram_tensor(name, shape, dt, kind="ExternalInput")
    out_d = nc.dram_tensor("out", [qg_pad, 32], f32, kind="ExternalOutput")

    with tile.TileContext(nc) as tc:
        with tc.tile_pool(name="const", bufs=1) as cp, \
             tc.tile_pool(name="ipool", bufs=8) as ip, \
             tc.tile_pool(name="gpool", bufs=8) as gp, \
             tc.tile_pool(name="tpool", bufs=3) as tp, \
             tc.tile_pool(name="epool", bufs=3) as ep, \
             tc.tile_pool(name="spool", bufs=3) as sp, \
             tc.tile_pool(name="zps", bufs=3, space="PSUM") as zps, \
             tc.tile_pool(name="hps", bufs=3, space="PSUM") as hps:
            ct = {}
            for name, shape, dt in [("wfull", [P, P], f16),
                                    ("w1b", [P, P], f16),
                                    ("c14", [P, 1], f32)]:
                ct[name] = cp.tile(shape, dt, tag=name, name=name + "_t")
                nc.sync.dma_start(out=ct[name][:], in_=consts[name][:])
            qxall = cp.tile([P, n_tiles * 32], f16, tag="qxall")
            nc.sync.dma_start(out=qxall[:], in_=qx_d[:])
            # per-tile 32-block transpose: xyz lands on partition rows 20-22
            qxT = cp.tile([P, n_tiles * 32], f16, tag="qxT")
            nc.vector.transpose(out=qxT[:], in_=qxall[:])
            out_stage = cp.tile([P, n_tiles * 32], f32)

            off16 = 0
            for gi, (ta, tb, nidx) in enumerate(groups):
                n16 = nidx // 16
                nblk = nidx // P
                idx_t = ip.tile([P, MAX_GATHER_IDX // 16], i16, tag="idx")
                nc.sync.dma_start(out=idx_t[:, :n16],
                                  in_=idx_d[:, off16:off16 + n16])
                off16 += n16
                g_t = gp.tile([P, max_blk * EL], f16, tag="G")
                nc.gpsimd.dma_gather(
                    out_ap=g_t[:, :nblk * EL].rearrange("p (c e) -> p c e",
                                                        e=EL),
                    in_ap=table_d[:],
                    idxs_ap=idx_t[:, :n16],
                    num_idxs=nidx,
                    num_idxs_reg=nidx,
                    elem_size=EL,
                    single_packet=False,
                    queue_num=gi % NQUEUES,
                )

                # pass 1: z matmul (+ query-term accumulate) + relu -> e16.
                # Batches are tile-aligned so the broadcast query tile is
                # constant within each batch.
                g3 = g_t[:].rearrange("p (s u) -> p s u", u=EL)
                e16 = ep.tile([P, max_blk * 32], f16, tag="e")
                kofs = 0
                for t in range(ta, tb):
                    K = int(ksched[t])
                    nb = (K + BATCH_SLOTS - 1) // BATCH_SLOTS
                    s0 = 0
                    for b in range(nb):
                        bs = (K - s0) // (nb - b)
                        o = kofs + s0
                        tr = tp.tile([P, BATCH_SLOTS * 32], f16, tag="tr")
                        nc.vector.transpose(
                            out=tr[:, :bs * 32].rearrange("p (s u) -> p s u",
                                                          u=32),
                            in_=g3[:, o:o + bs, 0:32])
                        psum_z = zps.tile([P, BATCH_SLOTS * 32], f32, tag="z")
                        nc.tensor.matmul(psum_z[:, :bs * 32],
                                         lhsT=ct["wfull"][:],
                                         rhs=tr[:, :bs * 32],
                                         start=True, stop=False)
                        qb = qxT[:, t * 32:(t + 1) * 32] \
                            .rearrange("p (k u) -> p k u", k=1) \
                            .to_broadcast([P, bs, 32])
                        nc.tensor.matmul(psum_z[:, :bs * 32],
                                         lhsT=ct["wfull"][:], rhs=qb,
                                         start=False, stop=True)
                        nc.scalar.activation(e16[:, o * 32:(o + bs) * 32],
                                             psum_z[:, :bs * 32],
                                             mybir.ActivationFunctionType.Relu)
                        s0 += bs
                    kofs += K

                # pass 2: h matmul + per-tile slot reduce
                kofs = 0
                for t in range(ta, tb):
                    K = int(ksched[t])
                    acc = sp.tile([P, 32], f32, tag="acc")
                    nb = (K + BATCH_SLOTS - 1) // BATCH_SLOTS
                    s0 = 0
                    for b in range(nb):
                        bs = (K - s0) // (nb - b)
                        psum_h = hps.tile([P, BATCH_SLOTS * 32], f32, tag="h")
                        nc.tensor.matmul(
                            psum_h[:, :bs * 32], lhsT=ct["w1b"][:],
                            rhs=e16[:, (kofs + s0) * 32:(kofs + s0 + bs) * 32],
                            start=True, stop=True)
                        bmax_in = psum_h[:, :bs * 32].rearrange(
                            "p (s f) -> p f s", s=bs)
                        if b == 0:
                            nc.vector.reduce_max(out=acc[:], in_=bmax_in,
                                                 axis=mybir.AxisListType.X)
                        else:
                            bmax = sp.tile([P, 32], f32, tag="bmax")
                            nc.vector.reduce_max(out=bmax[:], in_=bmax_in,
                                                 axis=mybir.AxisListType.X)
                            nc.vector.tensor_tensor(out=acc[:], in0=acc[:],
                                                    in1=bmax[:],
                                                    op=mybir.AluOpType.max)
                        s0 += bs
                    # bias + relu, then 32-block transpose to [128q, 32f]
                    outT = sp.tile([P, 32], f32, tag="outT")
                    nc.scalar.activation(outT[:], acc[:],
                                         mybir.ActivationFunctionType.Relu,
                                         bias=ct["c14"][:, 0:1])
                    nc.vector.transpose(
                        out=out_stage[:, t * 32:(t + 1) * 32], in_=outT[:])
                    kofs += K
                nc.sync.dma_start(
                    out=out_d[:].rearrange("(t p) f -> p t f", p=P)[:, ta:tb],
                    in_=out_stage[:, ta * 32:tb * 32]
                        .rearrange("p (t f) -> p t f", f=32))
    nc.finalize()
    return nc


def prepare(inputs):
    """Returns (nc, in_maps, postprocess)."""
    folded = _fold_weights(inputs)
    cores, meta = _plan(inputs)
    nc = _build_program(meta)
    in_maps = []
    for core in cores:
        idx_all, qx_img, tab = _build_core_arrays(core, meta, inputs)
        m = {"table": tab, "idx": idx_all, "qx": qx_img}
        m.update(folded)
        in_maps.append(m)

    def post(results):
        qg, n_q = meta["qg"], meta["n_q"]
        n_dummy = meta["n_dummy"]
        parts = []
        for ci, core in enumerate(cores):
            raw = np.asarray(results[ci]["out"])            # [qg_pad, 32]
            nq_local = core["nq_local"]
            partial = np.zeros((qg, 32), np.float32)
            partial[core["qrow"][n_dummy:]] = raw[n_dummy:]
            partial[core["deg"] == 0] = 0.0
            parts.append(partial[:nq_local])
        combined = [np.maximum(parts[2 * g], parts[2 * g + 1]) for g in range(4)]
        return np.concatenate(combined, axis=0).astype(np.float32)

    return nc, in_maps, post


def kernel(**inputs):
    from concourse.bass_utils import run_bass_kernel_spmd
    nc, in_maps, post = prepare(inputs)
    res = run_bass_kernel_spmd(nc, in_maps, core_ids=list(range(8)))
    return post(res.results)


# revision 8
# speedup vs baseline: 2.4502x; 1.0004x over previous
"""EdgeConv message-passing kernel for 8 Trainium2 NeuronCores.

Strategy (pair-split + padded slot schedule + multi-queue fp16 dma_gather):
  - Queries are range-partitioned into 4 groups; refs are split into 2 halves
    (so local table indices fit int16 for dma_gather). Core c handles query
    group c>>1 and ref half c&1: its edges are those with e_query in the
    group and e_ref in the half.
  - All BatchNorms are affine at inference and fold into the weights. The
    per-edge pre-relu feature is computed by ONE block-diagonal matmul:
        z = Wfull^T @ [ref_xyz | ref_feat | 1 | q_xyz | 0...]
    where the table rows carry [ref_xyz | ref_feat | 1] in fp16 and the
    query xyz is injected on-chip into the gathered columns (units 20-22).
        h = z_relu @ W1', out[q] = relu(max_e h + c1) (empty -> 0)
  - Per core, queries are degree-sorted into tiles of 128; each tile has
    K_t slots per query (K_t = max degree in tile; padding repeats a real
    edge - idempotent under max). Edge rows are fetched with dma_gather
    (256B fp16 elements) round-robin over 4 SWDGE queues so several
    gathers' DMA rings drain concurrently (~4x the serial gather rate).
  - On-chip per gather group: DVE 32x32 block-transposes put units on
    partitions in 4 query-blocks; the Wfull matmul + scalar relu produce
    e (fp16); the W1 matmul + strided reduce_max fold slots; bias+relu and
    a final block-transpose produce the [128, 32] output rows.
  - Host side does only data movement and trivial combines: input packing /
    permutation, inverse permutation of output rows, zeroing of empty
    segments, and the pairwise max between the two ref-halves of each query
    group (the all-reduce-max of per-shard segment_max partials).
"""
import numpy as np

import concourse.bass as bass
import concourse.tile as tile
from concourse import bacc, mybir

EPS = 1e-3
P = 128
EL = 128             # fp16 units per table row (256 B dma_gather element)
MAX_GATHER_IDX = 6144
BATCH_SLOTS = 16     # slots per psum batch (16*32 = 512 = one PSUM bank)
NQUEUES = 4


def _fold_weights(inputs):
    f = np.float32
    s0 = inputs["bn0_g"] / np.sqrt(inputs["bn0_v"] + EPS)
    t0 = inputs["bn0_b"] - inputs["bn0_m"] * s0
    sf = inputs["bnf_g"] / np.sqrt(inputs["bnf_v"] + EPS)
    tf = inputs["bnf_b"] - inputs["bnf_m"] * sf
    s1 = inputs["bn1_g"] / np.sqrt(inputs["bn1_v"] + EPS)
    t1 = inputs["bn1_b"] - inputs["bn1_m"] * s1

    # Wfull rows: 0-2 ref_xyz, 3-18 ref_feat, 19 const-1, 20-22 -query_xyz
    Wf = np.zeros((32, 32), f)
    Wf[0:3] = inputs["w_pos"] * s0
    Wf[3:19] = inputs["w_feat"] * sf
    Wf[19] = (t0 + tf).astype(f)
    Wf[20:23] = -(inputs["w_pos"] * s0)
    W1 = (inputs["w1"] * s1).astype(f)
    c1 = (inputs["b1"] * s1 + t1).astype(f)

    def bd(w, dtype):
        out = np.zeros((P, P), dtype)
        for b in range(4):
            out[b * 32:(b + 1) * 32, b * 32:(b + 1) * 32] = w
        return out

    return {
        "wfull": bd(Wf, np.float16),
        "w1b": bd(W1, np.float16),
        "c14": np.tile(c1, 4).reshape(P, 1).astype(f),
    }


def _plan(inputs):
    """Host-side partitioning: per-core edge slot schedules (int bookkeeping)."""
    e_ref = np.asarray(inputs["e_ref"]).astype(np.int64)
    e_query = np.asarray(inputs["e_query"]).astype(np.int64)
    n_ref = inputs["ref_bxyz"].shape[0]
    n_q = inputs["query_bxyz"].shape[0]
    half = (n_ref + 1) // 2
    assert half <= 32767, "local table indices must fit int16"
    qg = (n_q + 3) // 4                      # queries per group
    qg_pad = ((qg + P - 1) // P) * P
    n_tiles = qg_pad // P
    n_dummy = qg_pad - qg

    cores = []
    for g in range(4):
        qlo, qhi = g * qg, min((g + 1) * qg, n_q)
        for h in range(2):
            m = (e_query >= qlo) & (e_query < qhi) & \
                (e_ref >= h * half) & (e_ref < min((h + 1) * half, n_ref))
            er = (e_ref[m] - h * half).astype(np.int64)
            eq = (e_query[m] - qlo).astype(np.int64)
            nq_local = qhi - qlo
            deg = np.bincount(eq, minlength=qg)
            order = np.argsort(eq, kind="stable")
            er_s = er[order]
            ptr = np.zeros(qg + 1, np.int64)
            np.cumsum(deg, out=ptr[1:])
            perm = np.argsort(deg, kind="stable")      # ascending degree
            qrow = np.full(qg_pad, -1, np.int64)
            qrow[n_dummy:] = perm
            degrow = np.zeros(qg_pad, np.int64)
            degrow[n_dummy:] = deg[perm]
            ptrrow = np.zeros(qg_pad, np.int64)
            ptrrow[n_dummy:] = ptr[perm]
            kt = degrow.reshape(n_tiles, P).max(axis=1)
            kt = np.maximum(kt, 1)
            cores.append({
                "g": g, "h": h, "qlo": qlo, "nq_local": nq_local,
                "er_s": er_s, "deg": deg, "qrow": qrow,
                "degrow": degrow, "ptrrow": ptrrow, "kt": kt,
            })

    # shared slot schedule across the 8 SPMD cores
    ksched = np.max(np.stack([c["kt"] for c in cores]), axis=0)

    # gather groups: consecutive tiles, <= MAX_GATHER_IDX indices each.
    # The first NQUEUES groups are single tiles so all four SWDGE queues
    # spin up quickly instead of one full-size gather running solo.
    groups = []
    t = 0
    while t < n_tiles and len(groups) < NQUEUES:
        groups.append((t, t + 1, P * int(ksched[t])))
        t += 1
    t_hi = n_tiles - 2          # last two tiles: own groups (short tail)
    while t < t_hi:
        t0_, n = t, 0
        while t < t_hi and n + P * int(ksched[t]) <= MAX_GATHER_IDX:
            n += P * int(ksched[t])
            t += 1
        assert t > t0_, f"tile {t} alone exceeds MAX_GATHER_IDX"
        groups.append((t0_, t, n))
    while t < n_tiles:
        groups.append((t, t + 1, P * int(ksched[t])))
        t += 1

    meta = {
        "half": half, "qg": qg, "qg_pad": qg_pad, "n_tiles": n_tiles,
        "n_dummy": n_dummy, "ksched": ksched, "groups": groups, "n_q": n_q,
        "n_ref": n_ref,
    }
    return cores, meta


def _build_core_arrays(core, meta, inputs):
    """idx_all [128, TOT/16] int16, qx [128, n_tiles*3] f16, table [half, EL] f16."""
    half, qg_pad, n_tiles = meta["half"], meta["qg_pad"], meta["n_tiles"]
    ksched, groups = meta["ksched"], meta["groups"]
    n_ref = meta["n_ref"]
    er_s, degrow, ptrrow, qrow = (core["er_s"], core["degrow"],
                                  core["ptrrow"], core["qrow"])

    # per-tile [128, K] local table indices (pad: repeat edges cyclically)
    idx_tiles = []
    for t in range(n_tiles):
        rows = slice(t * P, (t + 1) * P)
        K = int(ksched[t])
        d = np.maximum(degrow[rows], 1)[:, None]
        j = np.arange(K)[None, :]
        pos = ptrrow[rows][:, None] + (j % d)
        if er_s.size:
            it = er_s[np.minimum(pos, er_s.size - 1)]
        else:
            it = np.zeros((P, K), np.int64)
        it = np.where(degrow[rows][:, None] > 0, it, 0)
        idx_tiles.append(it.astype(np.int16))

    # per gather group: flat index order j = s*128 + p (s = slot within group)
    wrapped = []
    for (ta, tb, nidx) in groups:
        blocks = np.concatenate([idx_tiles[t].T for t in range(ta, tb)], axis=0)
        flat = blocks.reshape(-1)                       # [nidx], j = s*128+p
        w = np.ascontiguousarray(flat.reshape(-1, 16).T)  # [16, nidx/16]
        wrapped.append(np.tile(w, (8, 1)))              # [128, nidx/16]
    idx_all = np.concatenate(wrapped, axis=1)

    # query xyz per tile-row, packed [128, n_tiles*32]: cols 20-22 of each
    # 32-col tile block carry xyz; the rest are zero (so the transposed tile
    # adds nothing on the table-data partition rows).
    qx = np.zeros((qg_pad, 32), np.float16)
    valid = qrow >= 0
    qx[valid, 20:23] = np.asarray(inputs["query_bxyz"])[core["qlo"] + qrow[valid],
                                                        1:4].astype(np.float16)
    qx_img = np.ascontiguousarray(
        qx.reshape(n_tiles, P, 32).transpose(1, 0, 2).reshape(P, n_tiles * 32))

    tab = np.zeros((half, EL), np.float16)
    lo = core["h"] * half
    hi = min(lo + half, n_ref)
    tab[:hi - lo, 0:3] = np.asarray(inputs["ref_bxyz"])[lo:hi, 1:4] \
        .astype(np.float16)
    tab[:hi - lo, 3:19] = np.asarray(inputs["ref_feat"])[lo:hi] \
        .astype(np.float16)
    tab[:hi - lo, 19] = 1.0
    return idx_all, qx_img, tab


def _build_program(meta):
    f32 = mybir.dt.float32
    f16 = mybir.dt.float16
    i16 = mybir.dt.int16
    half, qg_pad, n_tiles = meta["half"], meta["qg_pad"], meta["n_tiles"]
    ksched, groups = meta["ksched"], meta["groups"]
    tot16 = sum(n for (_, _, n) in groups) // 16
    max_blk = max(n for (_, _, n) in groups) // P

    nc = bacc.Bacc("TRN2", num_devices=8, num_swdge_queues=NQUEUES)
    table_d = nc.dram_tensor("table", [half, EL], f16, kind="ExternalInput")
    idx_d = nc.dram_tensor("idx", [P, tot16], i16, kind="ExternalInput")
    qx_d = nc.dram_tensor("qx", [P, n_tiles * 32], f16, kind="ExternalInput")
    consts = {}
    for name, shape, dt in [("wfull", [P, P], f16), ("w1b", [P, P], f16),
                            ("c14", [P, 1], f32)]:
        consts[name] = nc.dram_tensor(name, shape, dt, kind="ExternalInput")
    out_d = nc.dram_tensor("out", [qg_pad, 32], f32, kind="ExternalOutput")

    with tile.TileContext(nc) as tc:
        with tc.tile_pool(name="const", bufs=1) as cp, \
             tc.tile_pool(name="ipool", bufs=8) as ip, \
             tc.tile_pool(name="gpool", bufs=8) as gp, \
             tc.tile_pool(name="tpool", bufs=3) as tp, \
             tc.tile_pool(name="epool", bufs=3) as ep, \
             tc.tile_pool(name="spool", bufs=3) as sp, \
             tc.tile_pool(name="zps", bufs=3, space="PSUM") as zps, \
             tc.tile_pool(name="hps", bufs=3, space="PSUM") as hps:
            ct = {}
            for name, shape, dt in [("wfull", [P, P], f16),
                                    ("w1b", [P, P], f16),
                                    ("c14", [P, 1], f32)]:
                ct[name] = cp.tile(shape, dt, tag=name, name=name + "_t")
                nc.sync.dma_start(out=ct[name][:], in_=consts[name][:])
            qxall = cp.tile([P, n_tiles * 32], f16, tag="qxall")
            nc.sync.dma_start(out=qxall[:], in_=qx_d[:])
            # per-tile 32-block transpose: xyz lands on partition rows 20-22
            qxT = cp.tile([P, n_tiles * 32], f16, tag="qxT")
            nc.vector.transpose(out=qxT[:], in_=qxall[:])
            out_stage = cp.tile([P, n_tiles * 32], f32)

            off16 = 0
            for gi, (ta, tb, nidx) in enumerate(groups):
                n16 = nidx // 16
                nblk = nidx // P
                idx_t = ip.tile([P, MAX_GATHER_IDX // 16], i16, tag="idx")
                nc.sync.dma_start(out=idx_t[:, :n16],
                                  in_=idx_d[:, off16:off16 + n16])
                off16 += n16
                g_t = gp.tile([P, max_blk * EL], f16, tag="G")
                nc.gpsimd.dma_gather(
                    out_ap=g_t[:, :nblk * EL].rearrange("p (c e) -> p c e",
                                                        e=EL),
                    in_ap=table_d[:],
                    idxs_ap=idx_t[:, :n16],
                    num_idxs=nidx,
                    num_idxs_reg=nidx,
                    elem_size=EL,
                    single_packet=False,
                    queue_num=gi % NQUEUES,
                )

                # pass 1: z matmul (+ query-term accumulate) + relu -> e16.
                # Batches are tile-aligned so the broadcast query tile is
                # constant within each batch.
                g3 = g_t[:].rearrange("p (s u) -> p s u", u=EL)
                e16 = ep.tile([P, max_blk * 32], f16, tag="e")
                kofs = 0
                for t in range(ta, tb):
                    K = int(ksched[t])
                    nb = (K + BATCH_SLOTS - 1) // BATCH_SLOTS
                    s0 = 0
                    for b in range(nb):
                        bs = (K - s0) // (nb - b)
                        o = kofs + s0
                        tr = tp.tile([P, BATCH_SLOTS * 32], f16, tag="tr")
                        nc.vector.transpose(
                            out=tr[:, :bs * 32].rearrange("p (s u) -> p s u",
                                                          u=32),
                            in_=g3[:, o:o + bs, 0:32])
                        psum_z = zps.tile([P, BATCH_SLOTS * 32], f32, tag="z")
                        nc.tensor.matmul(psum_z[:, :bs * 32],
                                         lhsT=ct["wfull"][:],
                                         rhs=tr[:, :bs * 32],
                                         start=True, stop=False)
                        qb = qxT[:, t * 32:(t + 1) * 32] \
                            .rearrange("p (k u) -> p k u", k=1) \
                            .to_broadcast([P, bs, 32])
                        nc.tensor.matmul(psum_z[:, :bs * 32],
                                         lhsT=ct["wfull"][:], rhs=qb,
                                         start=False, stop=True)
                        nc.scalar.activation(e16[:, o * 32:(o + bs) * 32],
                                             psum_z[:, :bs * 32],
                                             mybir.ActivationFunctionType.Relu)
                        s0 += bs
                    kofs += K

                # pass 2: h matmul + per-tile slot reduce
                kofs = 0
                for t in range(ta, tb):
                    K = int(ksched[t])
                    acc = sp.tile([P, 32], f32, tag="acc")
                    nb = (K + BATCH_SLOTS - 1) // BATCH_SLOTS
                    s0 = 0
                    for b in range(nb):
                        bs = (K - s0) // (nb - b)
                        psum_h = hps.tile([P, BATCH_SLOTS * 32], f32, tag="h")
                        nc.tensor.matmul(
                            psum_h[:, :bs * 32], lhsT=ct["w1b"][:],
                            rhs=e16[:, (kofs + s0) * 32:(kofs + s0 + bs) * 32],
                            start=True, stop=True)
                        bmax_in = psum_h[:, :bs * 32].rearrange(
                            "p (s f) -> p f s", s=bs)
                        if b == 0:
                            nc.vector.reduce_max(out=acc[:], in_=bmax_in,
                                                 axis=mybir.AxisListType.X)
                        else:
                            bmax = sp.tile([P, 32], f32, tag="bmax")
                            nc.vector.reduce_max(out=bmax[:], in_=bmax_in,
                                                 axis=mybir.AxisListType.X)
                            nc.vector.tensor_tensor(out=acc[:], in0=acc[:],
                                                    in1=bmax[:],
                                                    op=mybir.AluOpType.max)
                        s0 += bs
                    # bias + relu, then 32-block transpose to [128q, 32f]
                    outT = sp.tile([P, 32], f32, tag="outT")
                    nc.scalar.activation(outT[:], acc[:],
                                         mybir.ActivationFunctionType.Relu,
                                         bias=ct["c14"][:, 0:1])
                    nc.vector.transpose(
                        out=out_stage[:, t * 32:(t + 1) * 32], in_=outT[:])
                    kofs += K
                nc.sync.dma_start(
                    out=out_d[:].rearrange("(t p) f -> p t f", p=P)[:, ta:tb],
                    in_=out_stage[:, ta * 32:tb * 32]
                        .rearrange("p (t f) -> p t f", f=32))
    nc.finalize()
    return nc


def prepare(inputs):
    """Returns (nc, in_maps, postprocess)."""
    folded = _fold_weights(inputs)
    cores, meta = _plan(inputs)
    nc = _build_program(meta)
    in_maps = []
    for core in cores:
        idx_all, qx_img, tab = _build_core_arrays(core, meta, inputs)
        m = {"table": tab, "idx": idx_all, "qx": qx_img}
        m.update(folded)
        in_maps.append(m)

    def post(results):
        qg, n_q = meta["qg"], meta["n_q"]
        n_dummy = meta["n_dummy"]
        parts = []
        for ci, core in enumerate(cores):
            raw = np.asarray(results[ci]["out"])            # [qg_pad, 32]
            nq_local = core["nq_local"]
            partial = np.zeros((qg, 32), np.float32)
            partial[core["qrow"][n_dummy:]] = raw[n_dummy:]
            partial[core["deg"] == 0] = 0.0
            parts.append(partial[:nq_local])
        combined = [np.maximum(parts[2 * g], parts[2 * g + 1]) for g in range(4)]
        return np.concatenate(combined, axis=0).astype(np.float32)

    return nc, in_maps, post


def kernel(**inputs):
    from concourse.bass_utils import run_bass_kernel_spmd
    nc, in_maps, post = prepare(inputs)
    res = run_bass_kernel_spmd(nc, in_maps, core_ids=list(range(8)))
    return post(res.results)
